# revision 1
# baseline (speedup 1.0000x reference)
"""Trainium2 Bass kernel for CubicalLayer gather_nd.

Problem: X[4096,4096] f32, indices[524288,2] int32 ->
         out[262144,2] f32, out.flat[k] = X[indices[k,0], indices[k,1]].

Strategy (data-parallel over the pair list, 8 NeuronCores):
  - Host shards the pair list by (row-stripe, column-phase): stripe = r//512
    picks the core (8MB window fits dma_gather's int16 index range), and
    phase = c%64 groups pairs so every gather chunk shares one within-block
    element offset. Each of the 64 phase classes is padded to 1152 slots.
  - Device (per core): computes int16 256B-block indices (r*64 + c//64) on
    the vector engine, bulk-gathers 64-float blocks from HBM with the SWDGE
    dma_gather custom instruction (72 chunks x 1024 indices, round-robin
    over 4 SWDGE queues), then extracts the target element of each block
    with a static strided copy (the phase is constant per class segment).
  - Host unshards: scatters per-core results back to original pair order.
"""

import numpy as np

import concourse.tile as tile
from concourse import bacc, mybir
from concourse.bass_utils import run_bass_kernel_spmd

H = 4096
W = 4096
N_IDX = 524288
NCORES = 8
P = 128

STRIPE_ROWS = H // NCORES  # 512
ELEM = 64  # f32 per gathered block (256B)
NPHASE = 64  # column phases (c % 64)
CLS = 1152  # padded slots per phase class (9*128; seed-0 max is 1119)
NPAD = NPHASE * CLS  # 73728 per core
GCHUNK = 1024  # indices per dma_gather instruction (SWDGE ring capacity)
LCHUNK = 4096  # indices per pair-load chunk
NQ = 4  # SWDGE queues
NCHUNKS = NPAD // GCHUNK  # 72
COLS = NPAD // P  # out free dim (576)


def build_kernel(reps=1):
    f16 = NPAD // 16
    n_lchunks = NPAD // LCHUNK  # 18
    gathers_per_l = LCHUNK // GCHUNK  # 4
    cg = GCHUNK // P  # groups per gather chunk (8)
    cls_g = CLS // P  # groups per class (9)

    nc = bacc.Bacc(
        "TRN2",
        target_bir_lowering=False,
        debug=False,
        num_devices=NCORES,
        num_swdge_queues=NQ,
    )
    XS = nc.dram_tensor("XS", [STRIPE_ROWS, W], mybir.dt.float32, kind="ExternalInput")
    # wrapped pair list: pair k at [k%16 (replicated x8), k//16, {r_local, c}]
    pairs = nc.dram_tensor("pairs", [P, f16, 2], mybir.dt.int32, kind="ExternalInput")
    out = nc.dram_tensor("out", [P, COLS], mybir.dt.float32, kind="ExternalOutput")

    xs_rows = XS.ap().rearrange("h (a b) -> (h a) b", b=ELEM)  # [32768, 64]

    with tile.TileContext(nc) as tc:
        with (
            tc.tile_pool(name="pairp", bufs=3) as pair_pool,
            tc.tile_pool(name="blkp", bufs=1) as blk_pool,
            tc.tile_pool(name="tmpp", bufs=3) as tmp_pool,
            tc.tile_pool(name="gp", bufs=8) as g_pool,
            tc.tile_pool(name="outp", bufs=1) as out_pool,
        ):
            blk16 = blk_pool.tile([P, f16], mybir.dt.int16)
            vals = out_pool.tile([P, COLS], mybir.dt.float32)

            with tc.For_i(0, reps, 1):
                for lc in range(n_lchunks):
                    fsl = slice(lc * (LCHUNK // 16), (lc + 1) * (LCHUNK // 16))
                    pt = pair_pool.tile([P, LCHUNK // 16, 2], mybir.dt.int32, tag="pt")
                    nc.sync.dma_start(out=pt[:, :, :], in_=pairs.ap()[:, fsl, :])
                    # blk = r*64 + (c >> 6), in [0, 32768)
                    b32 = tmp_pool.tile([P, LCHUNK // 16], mybir.dt.int32, tag="b32")
                    nc.vector.tensor_scalar(
                        out=b32[:, :],
                        in0=pt[:, :, 1],
                        scalar1=6,
                        scalar2=None,
                        op0=mybir.AluOpType.logical_shift_right,
                    )
                    b32b = tmp_pool.tile([P, LCHUNK // 16], mybir.dt.int32, tag="b32b")
                    nc.vector.tensor_scalar(
                        out=b32b[:, :],
                        in0=pt[:, :, 0],
                        scalar1=6,
                        scalar2=None,
                        op0=mybir.AluOpType.logical_shift_left,
                    )
                    nc.vector.tensor_tensor(
                        out=b32[:, :],
                        in0=b32[:, :],
                        in1=b32b[:, :],
                        op=mybir.AluOpType.add,
                    )
                    # cast to int16 (values < 32768)
                    nc.vector.tensor_copy(out=blk16[:, fsl], in_=b32[:, :])

                    for gi in range(gathers_per_l):
                        c = lc * gathers_per_l + gi
                        gsl = slice(c * (GCHUNK // 16), (c + 1) * (GCHUNK // 16))
                        g = g_pool.tile([P, cg, ELEM], mybir.dt.float32, tag="g")
                        nc.gpsimd.dma_gather(
                            out_ap=g[:, :, :],
                            in_ap=xs_rows,
                            idxs_ap=blk16[:, gsl],
                            num_idxs=GCHUNK,
                            num_idxs_reg=GCHUNK,
                            elem_size=ELEM,
                            queue_num=c % NQ,
                        )
                        # extract the phase element of each block: the chunk's
                        # group range [8c, 8c+8) intersects phase classes
                        # (CLS//128 = 9 groups each) at static boundaries.
                        g_lo = c * cg
                        while g_lo < (c + 1) * cg:
                            cls_idx = g_lo // cls_g
                            g_hi = min((cls_idx + 1) * cls_g, (c + 1) * cg)
                            phase = cls_idx  # classes shipped in phase order
                            nc.vector.tensor_copy(
                                out=vals[:, g_lo:g_hi],
                                in_=g[:, g_lo - c * cg : g_hi - c * cg, phase],
                            )
                            g_lo = g_hi

            nc.sync.dma_start(out=out.ap(), in_=vals[:, :])
    nc.compile()
    return nc


_NC_CACHE = {}


def _get_nc():
    if "nc" not in _NC_CACHE:
        _NC_CACHE["nc"] = build_kernel()
    return _NC_CACHE["nc"]


def _route(indices):
    """Host-side shard: route pair rows to (core, phase-class) slots."""
    r = indices[:, 0].astype(np.int64)
    c = indices[:, 1].astype(np.int64)
    key = (r >> 9) * NPHASE + (c & (NPHASE - 1))  # 512 classes
    # secondary sort by 256B-block index within each class: consecutive
    # gather descriptors then walk X in address order (DRAM row locality)
    blk = ((r & (STRIPE_ROWS - 1)) << 6) | (c >> 6)
    order = np.argsort(key * 32768 + blk, kind="stable")
    counts = np.bincount(key, minlength=NCORES * NPHASE)
    assert counts.max() <= CLS, f"class count {counts.max()} exceeds CLS={CLS}"
    starts = np.concatenate([[0], np.cumsum(counts)])
    in_maps = []
    gather_pos = []  # per core: (routed slot k -> original pair row) pairs
    for i in range(NCORES):
        rl = np.zeros(NPAD, np.int32)
        cc = np.zeros(NPAD, np.int32)
        slot_k = []
        pos_all = []
        for ph in range(NPHASE):
            cls_id = i * NPHASE + ph
            pos = order[starts[cls_id] : starts[cls_id + 1]]
            base = ph * CLS
            n = len(pos)
            rl[base : base + n] = (r[pos] - i * STRIPE_ROWS).astype(np.int32)
            cc[base : base + n] = c[pos].astype(np.int32)
            slot_k.append(base + np.arange(n))
            pos_all.append(pos)
        # wrapped [16, NPAD/16, 2], replicated x8 -> [128, NPAD/16, 2]
        pw = np.empty((16, NPAD // 16, 2), np.int32)
        pw[:, :, 0] = rl.reshape(NPAD // 16, 16).T
        pw[:, :, 1] = cc.reshape(NPAD // 16, 16).T
        pw = np.tile(pw, (8, 1, 1))
        in_maps.append({"pairs": pw})
        gather_pos.append(
            (np.concatenate(slot_k), np.concatenate(pos_all))
        )
    return in_maps, gather_pos


def kernel(X, indices):
    X = np.ascontiguousarray(np.asarray(X), dtype=np.float32)
    indices = np.asarray(indices, dtype=np.int32)
    nc = _get_nc()
    in_maps, gather_pos = _route(indices)
    for i in range(NCORES):
        in_maps[i]["XS"] = np.ascontiguousarray(
            X[i * STRIPE_ROWS : (i + 1) * STRIPE_ROWS]
        )
    res = run_bass_kernel_spmd(nc, in_maps, core_ids=list(range(NCORES)))
    out_flat = np.empty(N_IDX, np.float32)
    k = np.arange(NPAD)
    # routed slot k -> flat position in returned [P, COLS]:
    # value sits at vals[k%128, 8*(k//1024) + (k%1024)//128]
    land = (k % P) * COLS + 8 * (k // GCHUNK) + (k % GCHUNK) // P
    for i in range(NCORES):
        vals = res.results[i]["out"].reshape(-1)
        slot_k, pos = gather_pos[i]
        out_flat[pos] = vals[land[slot_k]]
    return out_flat.reshape(-1, 2)



# revision 2
# speedup vs baseline: 1.8147x; 1.8147x over previous
"""Trainium2 Bass kernel v3 for CubicalLayer gather_nd.

Problem: X[4096,4096] f32, indices[524288,2] int32 ->
         out[262144,2] f32, out.flat[k] = X[indices[k,0], indices[k,1]].

Strategy (8 NeuronCores, row-striped; ~65536 requests per core):
  - Host shards pairs by row stripe (core = r>>9) and phase class
    (ph = c%64, 64 classes padded to CLS=1152 slots) and precomputes the
    int16 256B-block index blk = (r%512)*64 + c//64 for each slot (the
    device does no index arithmetic; only the 1.2MB int16 index array is
    uploaded per pass instead of 4.7MB of int32 pairs).
  - Within a class, blocks are sorted ascending, rotated to start at DRAM
    quartile (ph%4)*8192 (the 4 concurrently-active SWDGE queues then walk
    disjoint address regions), and 16-way interleaved (concurrently-active
    DMA engines walk disjoint dense sub-runs). Measured on HW: address
    concentration collapses gather throughput ~15x, so padding slots
    recycle the class's own (spread) blocks rather than a constant.
  - Device: 72 SWDGE dma_gather ops per pass (1024 indices each, 256B
    blocks, round-robin over 4 queues); the vector engine extracts each
    class's phase element with static strided copies.
  - Host unshards: scatters per-core results back to original pair order.
"""

import numpy as np

import concourse.tile as tile
from concourse import bacc, mybir
from concourse.bass_utils import run_bass_kernel_spmd

H = 4096
W = 4096
N_IDX = 524288
NCORES = 8
P = 128

STRIPE_ROWS = H // NCORES  # 512
ELEM = 64  # f32 per gathered block (256B)
NPH = 64  # phase classes (c % 64)
CLS = 1152  # padded slots per class (9*128; binomial mean 1024, +4 sigma)
NPAD = NPH * CLS  # 73728
GCHUNK = 1024  # indices per dma_gather (HW SWDGE ring limit)
NQ = 4  # SWDGE queues
NCHUNKS = NPAD // GCHUNK  # 72
COLS = NPAD // P  # 576
NBLK = STRIPE_ROWS * W // ELEM  # 32768 blocks per stripe
IDXW = NPAD // 16  # 4608


def build_kernel(reps=1):
    cg = GCHUNK // P  # 8 groups per chunk
    cls_g = CLS // P  # 9 groups per class

    nc = bacc.Bacc(
        "TRN2",
        target_bir_lowering=False,
        debug=False,
        num_devices=NCORES,
        num_swdge_queues=NQ,
    )
    XS = nc.dram_tensor("XS", [STRIPE_ROWS, W], mybir.dt.float32, kind="ExternalInput")
    IDX = nc.dram_tensor("IDX", [P, IDXW], mybir.dt.int16, kind="ExternalInput")
    OUT = nc.dram_tensor("OUT", [P, COLS], mybir.dt.float32, kind="ExternalOutput")

    xs_rows = XS.ap().rearrange("h (a b) -> (h a) b", b=ELEM)  # [32768, 64]

    with tile.TileContext(nc) as tc:
        with (
            tc.tile_pool(name="ip", bufs=2) as i_pool,
            tc.tile_pool(name="gp", bufs=8) as g_pool,
            tc.tile_pool(name="op", bufs=1) as o_pool,
        ):
            vals = o_pool.tile([P, COLS], mybir.dt.float32)

            with tc.For_i(0, reps, 1):
                idx_sb = i_pool.tile([P, IDXW], mybir.dt.int16, tag="idx")
                nc.sync.dma_start(out=idx_sb[:, :], in_=IDX.ap())
                for c in range(NCHUNKS):
                    gsl = slice(c * (GCHUNK // 16), (c + 1) * (GCHUNK // 16))
                    g = g_pool.tile([P, cg, ELEM], mybir.dt.float32, tag="g")
                    nc.gpsimd.dma_gather(
                        out_ap=g[:, :, :],
                        in_ap=xs_rows,
                        idxs_ap=idx_sb[:, gsl],
                        num_idxs=GCHUNK,
                        num_idxs_reg=GCHUNK,
                        elem_size=ELEM,
                        queue_num=c % NQ,
                    )
                    # extract the phase element: the chunk's group range
                    # [8c, 8c+8) intersects classes (9 groups each) at
                    # static boundaries
                    g_lo = c * cg
                    while g_lo < (c + 1) * cg:
                        cls_idx = g_lo // cls_g
                        g_hi = min((cls_idx + 1) * cls_g, (c + 1) * cg)
                        nc.vector.tensor_copy(
                            out=vals[:, g_lo:g_hi],
                            in_=g[:, g_lo - c * cg : g_hi - c * cg, cls_idx],
                        )
                        g_lo = g_hi

            nc.sync.dma_start(out=OUT.ap(), in_=vals[:, :])
    nc.compile()
    return nc


_NC_CACHE = {}


def _get_nc():
    if "nc" not in _NC_CACHE:
        _NC_CACHE["nc"] = build_kernel()
    return _NC_CACHE["nc"]


def _route(indices):
    """Host-side shard: route pairs to (core, class) slots, build the int16
    block-index array per core and the slot->pair map."""
    r = indices[:, 0].astype(np.int64)
    c = indices[:, 1].astype(np.int64)
    core = r >> 9
    ph = c & (NPH - 1)
    blk = ((r & (STRIPE_ROWS - 1)) << 6) | (c >> 6)  # [0, 32768)
    key = core * NPH + ph  # 512 bins
    order = np.argsort(key * (NBLK + 1) + blk, kind="stable")
    counts = np.bincount(key, minlength=NCORES * NPH)
    assert counts.max() <= CLS, f"class count {counts.max()} exceeds CLS={CLS}"
    starts = np.concatenate([[0], np.cumsum(counts)])

    grids = {}

    def grid_for(n):
        if n not in grids:
            L = (n + 15) // 16
            gr = np.arange(16 * L).reshape(16, L).T.reshape(-1)
            grids[n] = gr[gr < n]
        return grids[n]

    in_maps = []
    gather_pos = []
    for i in range(NCORES):
        idx_arr = np.zeros(NPAD, np.int16)
        slot_all = []
        rows_all = []
        for p in range(NPH):
            b = i * NPH + p
            seg = order[starts[b] : starts[b + 1]]  # pair rows, blk ascending
            n = len(seg)
            base = p * CLS
            if n == 0:
                idx_arr[base : base + CLS] = (p * 512) % NBLK
                continue
            sb = blk[seg]
            cut = np.searchsorted(sb, (p % NQ) * (NBLK // NQ))
            rot = np.concatenate([np.arange(cut, n), np.arange(cut)])
            perm = rot[grid_for(n)]
            vals16 = sb[perm].astype(np.int16)
            idx_arr[base : base + n] = vals16
            # recycle the class's own spread blocks into the padding slots
            idx_arr[base + n : base + CLS] = np.resize(vals16, CLS - n)
            slot_all.append(base + np.arange(n))
            rows_all.append(seg[perm])
        iw = np.tile(idx_arr.reshape(NPAD // 16, 16).T, (8, 1))
        in_maps.append({"IDX": iw})
        gather_pos.append((np.concatenate(slot_all), np.concatenate(rows_all)))
    return in_maps, gather_pos


def kernel(X, indices):
    X = np.ascontiguousarray(np.asarray(X), dtype=np.float32)
    indices = np.asarray(indices, dtype=np.int32)
    nc = _get_nc()
    in_maps, gather_pos = _route(indices)
    for i in range(NCORES):
        in_maps[i]["XS"] = np.ascontiguousarray(
            X[i * STRIPE_ROWS : (i + 1) * STRIPE_ROWS]
        )
    res = run_bass_kernel_spmd(nc, in_maps, core_ids=list(range(NCORES)))
    out_flat = np.empty(N_IDX, np.float32)
    k = np.arange(NPAD)
    # routed slot k -> flat position in [P, COLS]: vals[k%128, k//128]
    land = (k % P) * COLS + k // P
    for i in range(NCORES):
        vals = res.results[i]["OUT"].reshape(-1)
        slots, rows = gather_pos[i]
        out_flat[rows] = vals[land[slots]]
    return out_flat.reshape(-1, 2)
